# revision 18
# baseline (speedup 1.0000x reference)
"""FPN ROI-align (crop_and_resize over 5 levels, 7x7 bilinear, channel concat)
on 8 Trainium2 NeuronCores, data-parallel over the batch dim (1 batch/core).

Self-contained: hardcodes shapes B=8, M=100, C=256, fm sizes 128/64/32/16/8,
strides 8/16/32/64/128, POOL=7.

Strategy per core:
  - compute per-box per-level sample coords/weights on device ([100,7] tiles)
  - expand to per-pixel [100,49] arrays, round-trip through DRAM scratch to
    re-layout as [128, 39] (pixel g = p*39+c)
  - per level, per chunk c: indirect-DMA gather 2 indices/pixel (each index
    fetches a contiguous 2-pixel row pair = 512 f32), bilinear-combine with
    4 per-partition scalar weights (validity pre-folded), store to rois.
"""
import sys

sys.path.insert(0, "/opt/trn_rl_repo")

import numpy as np
from contextlib import ExitStack

import concourse.bass as bass
import concourse.tile as tile
from concourse import bacc, mybir

F32 = mybir.dt.float32
I32 = mybir.dt.int32
AL = mybir.AluOpType

B, M, C = 8, 100, 256
POOL = 7
SIZES = (128, 64, 32, 16, 8)
STRIDES = (8, 16, 32, 64, 128)
NPIX = M * POOL * POOL  # 4900
NCHUNK = 39  # pixel g = p*39 + c, p in [0,128), c in [0,39); 128*39=4992 >= 4900


def build_nc():
    nc = bacc.Bacc()

    boxes = nc.declare_dram_parameter("boxes", [M, 5], F32, isOutput=False)
    fms = [
        nc.declare_dram_parameter(f"fm{l}", [SIZES[l] * SIZES[l], C], F32, isOutput=False)
        for l in range(5)
    ]
    rois = nc.declare_dram_parameter("rois", [NPIX, 5 * C], F32, isOutput=True)

    with tile.TileContext(nc) as tc, ExitStack() as ctx:
        prep = ctx.enter_context(tc.tile_pool(name="prep", bufs=1))
        persist = ctx.enter_context(tc.tile_pool(name="persist", bufs=1))
        dscr = ctx.enter_context(tc.tile_pool(name="dscr", bufs=1, space="DRAM"))
        gpool = ctx.enter_context(tc.tile_pool(name="gpool", bufs=6))
        opool = ctx.enter_context(tc.tile_pool(name="opool", bufs=4))

        # ---- load boxes, build t = arange(7)/6 ----
        bx = prep.tile([M, 5], F32, tag="bx")
        nc.sync.dma_start(bx[:], boxes[:])
        ti = prep.tile([M, POOL], I32, tag="ti")
        nc.gpsimd.iota(ti[:], pattern=[[1, POOL]], base=0, channel_multiplier=0)
        tf = prep.tile([M, POOL], F32, tag="tf")
        nc.vector.tensor_copy(tf[:], ti[:])
        nc.vector.tensor_scalar(tf[:], tf[:], 1.0 / 6.0, None, AL.mult)

        x1 = bx[:, 0:1]
        y1 = bx[:, 1:2]
        x2 = bx[:, 2:3]
        y2 = bx[:, 3:4]
        dx = prep.tile([M, 1], F32, tag="dx")
        dy = prep.tile([M, 1], F32, tag="dy")
        nc.vector.tensor_tensor(out=dx[:], in0=x2, in1=x1, op=AL.subtract)
        nc.vector.tensor_tensor(out=dy[:], in0=y2, in1=y1, op=AL.subtract)

        tb = tf[:]

        zf = prep.tile([1, 128 * NCHUNK - NPIX], F32, tag="zf")
        zi = prep.tile([1, 128 * NCHUNK - NPIX], I32, tag="zi")
        nc.vector.memset(zf[:], 0.0)
        nc.vector.memset(zi[:], 0)

        idx2s, wquads = [], []
        for l in range(5):
            S = SIZES[l]
            s = STRIDES[l]
            inv_s = 1.0 / s
            HW2 = S * S - 2

            def axis_prep(lo_ap, d_ap, name):
                # xs = (lo + t*d) / s  -> [M, POOL]
                los = prep.tile([M, 1], F32, tag=f"los{name}{l}")
                ds_ = prep.tile([M, 1], F32, tag=f"ds{name}{l}")
                nc.vector.tensor_scalar(los[:], lo_ap, inv_s, None, AL.mult)
                nc.vector.tensor_scalar(ds_[:], d_ap, inv_s, None, AL.mult)
                xs = prep.tile([M, POOL], F32, tag=f"xs{name}{l}")
                nc.vector.tensor_scalar(xs[:], tb, ds_[:, 0:1], los[:, 0:1], AL.mult, AL.add)
                # floor(xs) robust to any f32->int32 rounding mode:
                # xi = int(xs); xf = float(xi); x0 = xf - (xf > xs)
                xi = prep.tile([M, POOL], I32, tag=f"xi{name}{l}")
                nc.vector.tensor_copy(xi[:], xs[:])
                xf = prep.tile([M, POOL], F32, tag=f"xf{name}{l}")
                nc.vector.tensor_copy(xf[:], xi[:])
                tb_ = prep.tile([M, POOL], F32, tag=f"tbg{name}{l}")
                nc.vector.tensor_tensor(out=tb_[:], in0=xf[:], in1=xs[:], op=AL.is_gt)
                x0 = prep.tile([M, POOL], F32, tag=f"x0{name}{l}")
                nc.vector.tensor_tensor(out=x0[:], in0=xf[:], in1=tb_[:], op=AL.subtract)
                w = prep.tile([M, POOL], F32, tag=f"w{name}{l}")
                nc.vector.tensor_tensor(out=w[:], in0=xs[:], in1=x0[:], op=AL.subtract)
                v = prep.tile([M, POOL], F32, tag=f"v{name}{l}")
                nc.vector.tensor_scalar(v[:], xs[:], float(S - 1), None, AL.is_le)
                onemw = prep.tile([M, POOL], F32, tag=f"onemw{name}{l}")
                nc.vector.tensor_scalar(onemw[:], w[:], -1.0, 1.0, AL.mult, AL.add)
                q1 = prep.tile([M, POOL], F32, tag=f"q1{name}{l}")
                q2 = prep.tile([M, POOL], F32, tag=f"q2{name}{l}")
                nc.vector.tensor_tensor(out=q1[:], in0=onemw[:], in1=v[:], op=AL.mult)
                nc.vector.tensor_tensor(out=q2[:], in0=w[:], in1=v[:], op=AL.mult)
                x0c = prep.tile([M, POOL], F32, tag=f"x0c{name}{l}")
                nc.vector.tensor_scalar(x0c[:], x0[:], float(S - 1), None, AL.min)
                return xs, q1, q2, x0c

            _, q1x, q2x, x0c = axis_prep(x1, dx[:], "x")
            _, p1y, p2y, y0c = axis_prep(y1, dy[:], "y")
            rowf = prep.tile([M, POOL], F32, tag=f"rowf{l}")
            nc.vector.tensor_scalar(rowf[:], y0c[:], float(S), None, AL.mult)

            # ---- expand to [M, 49]: out[n, i*7+j] = I[n,i] op J[n,j] ----
            def iview(t):
                a = t[:]
                return bass.AP(a.tensor, a.offset, [a.ap[0], [1, POOL], [0, POOL]])

            def jview(t):
                a = t[:]
                return bass.AP(a.tensor, a.offset, [a.ap[0], [0, POOL], [1, POOL]])

            wexp = prep.tile([M, 4, 49], F32, tag=f"wexp{l}")
            wv = wexp[:].rearrange("p t (i j) -> p t i j", i=POOL)
            # a=(1-wy)(1-wx)vyvx, b=(1-wy)wx, c=wy(1-wx), d=wywx
            nc.vector.tensor_tensor(out=wv[:, 0], in0=iview(p1y), in1=jview(q1x), op=AL.mult)
            nc.vector.tensor_tensor(out=wv[:, 1], in0=iview(p1y), in1=jview(q2x), op=AL.mult)
            nc.vector.tensor_tensor(out=wv[:, 2], in0=iview(p2y), in1=jview(q1x), op=AL.mult)
            nc.vector.tensor_tensor(out=wv[:, 3], in0=iview(p2y), in1=jview(q2x), op=AL.mult)

            itf = prep.tile([M, 2, 49], F32, tag=f"itf{l}")
            itv = itf[:].rearrange("p t (i j) -> p t i j", i=POOL)
            nc.vector.tensor_tensor(out=itv[:, 0], in0=iview(rowf), in1=jview(x0c), op=AL.add)
            nc.vector.tensor_scalar(itf[:, 0], itf[:, 0], float(HW2), None, AL.min)
            nc.vector.tensor_scalar(itf[:, 1], itf[:, 0], float(S), float(HW2), AL.add, AL.min)
            iti = prep.tile([M, 2, 49], I32, tag=f"iti{l}")
            nc.vector.tensor_copy(iti[:], itf[:])

            # ---- DRAM round-trip to pixel-major [128, 39] layout ----
            wscr = [dscr.tile([1, 128 * NCHUNK], F32, tag=f"wscr{l}_{t}", name=f"wscr{l}_{t}") for t in range(4)]
            iscr = [dscr.tile([1, 128 * NCHUNK], I32, tag=f"iscr{l}_{t}", name=f"iscr{l}_{t}") for t in range(2)]
            for t in range(4):
                nc.sync.dma_start(wscr[t][:, 0:NPIX], wexp[:, t])
                nc.sync.dma_start(wscr[t][:, NPIX:], zf[:])
            for t in range(2):
                nc.sync.dma_start(iscr[t][:, 0:NPIX], iti[:, t])
                nc.sync.dma_start(iscr[t][:, NPIX:], zi[:])

            wquad = persist.tile([128, 4, NCHUNK], F32, tag=f"wquad{l}")
            idx2 = persist.tile([128, 2, NCHUNK], I32, tag=f"idx2{l}")
            for t in range(4):
                nc.sync.dma_start(
                    wquad[:, t, :],
                    wscr[t][0].rearrange("(p c) -> p c", p=128),
                )
            for t in range(2):
                nc.sync.dma_start(
                    idx2[:, t, :],
                    iscr[t][0].rearrange("(p c) -> p c", p=128),
                )
            # clamp garbage tail (rows beyond NPIX) into safe index range
            nc.vector.tensor_scalar(
                idx2[:].rearrange("p t c -> p (t c)"),
                idx2[:].rearrange("p t c -> p (t c)"),
                0,
                HW2,
                AL.max,
                AL.min,
            )
            idx2s.append(idx2)
            wquads.append(wquad)

        # ---- main loop ----
        for l in range(5):
            fm = fms[l]
            idx2 = idx2s[l]
            wquad = wquads[l]
            for c in range(NCHUNK):
                pv = 126 if c <= 24 else 125  # valid partitions in this chunk
                Gt = gpool.tile([128, 2 * C], F32, tag="Gt")
                Gb = gpool.tile([128, 2 * C], F32, tag="Gb")
                nc.gpsimd.indirect_dma_start(
                    out=Gt[:],
                    out_offset=None,
                    in_=fm[:],
                    in_offset=bass.IndirectOffsetOnAxis(ap=idx2[:, 0, c : c + 1], axis=0),
                )
                nc.gpsimd.indirect_dma_start(
                    out=Gb[:],
                    out_offset=None,
                    in_=fm[:],
                    in_offset=bass.IndirectOffsetOnAxis(ap=idx2[:, 1, c : c + 1], axis=0),
                )
                tl, tr = Gt[:, 0:C], Gt[:, C : 2 * C]
                bl, br = Gb[:, 0:C], Gb[:, C : 2 * C]
                o = opool.tile([128, C], F32, tag="o")
                t1 = opool.tile([128, C], F32, tag="t1")
                t2 = opool.tile([128, C], F32, tag="t2")
                t3 = opool.tile([128, C], F32, tag="t3")
                nc.scalar.mul(o[:], tl, wquad[:, 0, c : c + 1])
                nc.scalar.mul(t1[:], tr, wquad[:, 1, c : c + 1])
                nc.vector.tensor_scalar(t2[:], bl, wquad[:, 2, c : c + 1], None, AL.mult)
                nc.vector.tensor_scalar(t3[:], br, wquad[:, 3, c : c + 1], None, AL.mult)
                nc.vector.tensor_tensor(out=o[:], in0=o[:], in1=t1[:], op=AL.add)
                nc.vector.tensor_tensor(out=t2[:], in0=t2[:], in1=t3[:], op=AL.add)
                nc.vector.tensor_tensor(out=o[:], in0=o[:], in1=t2[:], op=AL.add)
                # store rows g = p*39 + c, channel block l
                dst = bass.AP(
                    rois[:].tensor,
                    c * (5 * C) + l * C,
                    [[NCHUNK * 5 * C, pv], [1, C]],
                )
                nc.sync.dma_start(dst, o[0:pv, :])

    nc.finalize()
    return nc


_NC = None


def _get_nc():
    global _NC
    if _NC is None:
        _NC = build_nc()
    return _NC


def prepare_in_maps(gt_boxes, fm0, fm1, fm2, fm3, fm4):
    gt_boxes = np.asarray(gt_boxes)
    fms_np = [np.asarray(f) for f in (fm0, fm1, fm2, fm3, fm4)]
    in_maps = []
    for b in range(B):
        m = {"boxes": gt_boxes[b].astype(np.float32)}
        for l in range(5):
            S = SIZES[l]
            m[f"fm{l}"] = np.ascontiguousarray(fms_np[l][b].reshape(S * S, C).astype(np.float32))
        in_maps.append(m)
    return in_maps


def kernel(gt_boxes, fm0, fm1, fm2, fm3, fm4):
    from concourse.bass_utils import run_bass_kernel_spmd

    nc = _get_nc()
    in_maps = prepare_in_maps(gt_boxes, fm0, fm1, fm2, fm3, fm4)
    res = run_bass_kernel_spmd(nc, in_maps, list(range(B))).results
    rois = np.stack([res[b]["rois"] for b in range(B)], axis=0)  # [B, 4900, 1280]
    rois = rois.reshape(B * M, POOL, POOL, 5 * C)
    box_ids = np.tile(np.arange(B, dtype=np.int32)[:, None], (1, M)).reshape(-1)
    return rois, box_ids
